# revision 25
# baseline (speedup 1.0000x reference)
"""Trainium2 Bass kernel for nn_NonSpikingOutput.

Reference semantics (N=4096 neurons, O=3 outputs, T=4096 steps):
    g = k/(e-k); act = clip(u, 0, 1); RK2 with i_syn frozen collapses to
        v_t = a_t * v_{t-1} + b_t
        a_t = 0.625 - 0.075*act*g,  b_t = 0.075*act*g*e = (0.625 - a_t)*e
    out[o, t] = sum_n v[n, o, t]

Final design (v8; every choice below is HW-measured, see git-less backups
kernel_v3/v4/v6_backup.py for the ancestry — 319us -> 253 -> 236us):
  - Inputs uploaded as bf16 (host truncation): halves HBM traffic and makes
    every DVE tensor_tensor eligible for the 2x perf mode.
  - d = e-k computed on the PE: psum_d = I@e + (-I)@k (identity stationaries
    uploaded as host constants). ACT Ln reads the f32 psum directly.
  - 0.075 folded into the ACT Exp bias: h = exp(-ln(e-k) + ln 0.075)
    = 0.075/(e-k), so c = act*k*h, a = 0.625 - c (ACT Copy), b = c*e (DVE),
    and the scan yields v directly -- no rescale.
  - DVE carries only: clip (TS 4x), t=k*h, c=t*act, b=c*e (TT 2x bf16), scan.
  - Software pipelining: scan/carry/colsum for tile i-1 are emitted during
    tile i, so the c(i)->a(i)->scan(i) ACT round trip never stalls DVE.
  - PSUM: d tile (128,2048)f32 = 4 banks (bufs=1) + colsum row (1,2048)f32 =
    4 banks (bufs=1) -- exactly 8 banks.
  - No GPSIMD (concurrent GPSIMD inflates DVE TT 4.5x via SBUF port sharing).

Measured engine budget per core (236us wall): DVE 200us busy (scan 106.8 +
3 TT 87.5 + clips 5.5 -- all at their per-mode throughput ceilings), ACT
171us, PE 193us, DMA ~100us. Rejected variants (measured worse): scan/a
halving (+overhead > stall savings), u-prefetch (DMA-bandwidth bound),
STT fusion (no DVE fast mode), GPSIMD offload (poisons DVE).

Sharding: neuron dim N split across 8 cores (512 each); host sums the
per-core (O, T) partials.
"""

import sys
from contextlib import ExitStack

import numpy as np

sys.path.insert(0, "/opt/trn_rl_repo")

import concourse.bass as bass
import concourse.tile as tile
from concourse import bacc, mybir
from concourse.bass_utils import run_bass_kernel_spmd

N_CORES = 8
N, O, T = 4096, 3, 4096
NL = N // N_CORES  # neurons per core
NG = NL // 128     # 128-partition neuron groups per core
F = 2048           # time-chunk (free dim) per tile
TC = T // F
FP32 = mybir.dt.float32
BF16 = mybir.dt.bfloat16
OP = mybir.AluOpType
AF = mybir.ActivationFunctionType

LN_0075 = float(np.log(0.075))  # Exp bias: exp(-lnd + ln 0.075) = 0.075/d


def _build_nc() -> bass.Bass:
    nc = bacc.Bacc(
        "TRN2", target_bir_lowering=False, debug=False, num_devices=N_CORES
    )
    u = nc.dram_tensor("u", [NL, T], BF16, kind="ExternalInput")
    k = nc.dram_tensor("k", [NL, O, T], BF16, kind="ExternalInput")
    e = nc.dram_tensor("e", [NL, O, T], BF16, kind="ExternalInput")
    ident_d = nc.dram_tensor("ident", [128, 128], BF16, kind="ExternalInput")
    nident_d = nc.dram_tensor("nident", [128, 128], BF16, kind="ExternalInput")
    out = nc.dram_tensor("out", [O, T], FP32, kind="ExternalOutput")

    with tile.TileContext(nc) as tc, ExitStack() as ctx:
        # Preload the ACT table set holding Ln+Exp+Copy (set 6) once.
        preload = mybir.InstLoadActFuncSet(
            name=nc.get_next_instruction_name(), act_func_set_id=6, ins=[], outs=[]
        )
        nc.scalar.add_instruction(preload)

        const_pool = ctx.enter_context(tc.tile_pool(name="const", bufs=1))
        ones = const_pool.tile([128, 1], BF16)
        nc.vector.memset(ones[:], 1.0)
        exp_bias = const_pool.tile([128, 1], FP32)
        nc.vector.memset(exp_bias[:], LN_0075)
        ident = const_pool.tile([128, 128], BF16)
        nc.sync.dma_start(ident[:], ident_d[:, :])
        nident = const_pool.tile([128, 128], BF16)
        nc.sync.dma_start(nident[:], nident_d[:, :])
        # one carry column per (o, g): column o*NG+g
        carry = const_pool.tile([128, O * NG], FP32)

        u_pool = ctx.enter_context(tc.tile_pool(name="u", bufs=2))
        act_pool = ctx.enter_context(tc.tile_pool(name="act", bufs=NG + 1))
        k_pool = ctx.enter_context(tc.tile_pool(name="k", bufs=3))
        e_pool = ctx.enter_context(tc.tile_pool(name="e", bufs=3))
        l_pool = ctx.enter_context(tc.tile_pool(name="l", bufs=3))
        h_pool = ctx.enter_context(tc.tile_pool(name="h", bufs=3))
        t_pool = ctx.enter_context(tc.tile_pool(name="t", bufs=3))
        c_pool = ctx.enter_context(tc.tile_pool(name="c", bufs=3))
        a_pool = ctx.enter_context(tc.tile_pool(name="a", bufs=3))
        b_pool = ctx.enter_context(tc.tile_pool(name="b", bufs=3))
        w_pool = ctx.enter_context(tc.tile_pool(name="w", bufs=3))
        r_pool = ctx.enter_context(tc.tile_pool(name="r", bufs=2))
        ps_pool = ctx.enter_context(tc.tile_pool(name="ps", bufs=1, space="PSUM"))
        d_pool = ctx.enter_context(tc.tile_pool(name="d", bufs=1, space="PSUM"))

        acts: dict[int, object] = {}
        ps_by_to: dict[tuple, object] = {}
        pending = None  # (tci, o, g, at, bt)
        evac_stash: list = []  # [(tci, o, ps)] deferred one tile

        def flush_evac():
            """Deferred psum evacuation: emitted one tile after the group
            closes so the a-Copy of the current tile precedes it in ACT's
            in-order queue (evac ahead of `a` stalled the scan ~2us per
            group boundary). It still precedes the next ps alloc (bufs=1)."""
            while evac_stash:
                tci, o, ps = evac_stash.pop(0)
                t0 = tci * F
                row = r_pool.tile([1, F], FP32, tag="row")
                nc.scalar.copy(row[:], ps[:])
                nc.sync.dma_start(out[o : o + 1, t0 : t0 + F], row[:, :])

        def emit_tail(item):
            """scan + carry + colsum for a finished front-end tile."""
            tci, o, g, at, bt, S = item
            wt = w_pool.tile([128, F], BF16, tag="w")
            ci = o * NG + g
            init = 0.0 if tci == 0 else carry[:, ci : ci + 1]
            C = F // S
            for s in range(S):
                sl = slice(s * C, (s + 1) * C)
                sub_init = init if s == 0 else wt[:, s * C - 1 : s * C]
                nc.vector.tensor_tensor_scan(
                    wt[:, sl], at[:, sl], bt[:, sl], sub_init, OP.mult, OP.add
                )
            if tci < TC - 1:
                nc.scalar.copy(carry[:, ci : ci + 1], wt[:, F - 1 : F])
            flush_evac()
            if g == 0:
                ps_by_to[(tci, o)] = ps_pool.tile(
                    [1, F], FP32, tag="ps", name=f"ps{tci}_{o}"
                )
            ps = ps_by_to[(tci, o)]
            for s in range(F // 512):
                nc.tensor.matmul(
                    ps[0:1, s * 512 : (s + 1) * 512],
                    ones[:],
                    wt[:, s * 512 : (s + 1) * 512],
                    start=(g == 0),
                    stop=(g == NG - 1),
                )
            if g == NG - 1:
                evac_stash.append((tci, o, ps))

        for tci in range(TC):
            t0 = tci * F
            for o in range(O):
                for g in range(NG):
                    p0 = g * 128
                    kt = k_pool.tile([128, F], BF16, tag="k")
                    et = e_pool.tile([128, F], BF16, tag="e")
                    dma_chunks = {(0, 0, 0): 4, (0, 0, 1): 2}.get((tci, o, g), 1)
                    DC = F // dma_chunks
                    for s in range(dma_chunks):
                        sl = slice(s * DC, (s + 1) * DC)
                        nc.sync.dma_start(kt[:, sl], k[p0 : p0 + 128, o, t0 + s * DC : t0 + (s + 1) * DC])
                        nc.sync.dma_start(et[:, sl], e[p0 : p0 + 128, o, t0 + s * DC : t0 + (s + 1) * DC])

                    # u/clip after k/e: the k,e -> PE -> Ln -> Exp chain is the
                    # critical path; the clip is DVE filler work.
                    if o == 0:
                        ut = u_pool.tile([128, F], BF16, tag="u")
                        nc.sync.dma_start(ut[:], u[p0 : p0 + 128, t0 : t0 + F])
                        av = act_pool.tile([128, F], BF16, tag="act")
                        nc.vector.tensor_scalar(av[:], ut[:], 0.0, 1.0, OP.max, OP.min)
                        acts[g] = av
                    act = acts[g]

                    # d = e - k on the PE: per 512-chunk, I@e then (-I)@k
                    dps = d_pool.tile([128, F], FP32, tag="d", name=f"d{tci}_{o}_{g}")
                    for s in range(F // 512):
                        sl = slice(s * 512, (s + 1) * 512)
                        nc.tensor.matmul(
                            dps[:, sl], ident[:], et[:, sl], start=True, stop=False
                        )
                        nc.tensor.matmul(
                            dps[:, sl], nident[:], kt[:, sl], start=False, stop=True
                        )

                    # Prime tile: sub-chunk the whole chain so the first DVE
                    # work starts ~7us earlier (pipeline warmup is otherwise
                    # gated on full-tile PE d + Ln + Exp of tile 0).
                    S = {(0, 0, 0): 4, (0, 0, 1): 2}.get((tci, o, g), 1)
                    C = F // S
                    lnd = l_pool.tile([128, F], FP32, tag="lnd")
                    ht = h_pool.tile([128, F], BF16, tag="h")
                    tt = t_pool.tile([128, F], BF16, tag="t")
                    ct = c_pool.tile([128, F], BF16, tag="c")
                    at = a_pool.tile([128, F], BF16, tag="a")
                    bt = b_pool.tile([128, F], BF16, tag="b")
                    for s in range(S):
                        sl = slice(s * C, (s + 1) * C)
                        nc.scalar.activation(lnd[:, sl], dps[:, sl], AF.Ln)
                        nc.scalar.activation(
                            ht[:, sl], lnd[:, sl], AF.Exp, bias=exp_bias[:], scale=-1.0
                        )
                        nc.vector.tensor_tensor(tt[:, sl], kt[:, sl], ht[:, sl], OP.mult)
                        nc.vector.tensor_tensor(ct[:, sl], tt[:, sl], act[:, sl], OP.mult)
                        nc.scalar.activation(
                            at[:, sl], ct[:, sl], AF.Copy, bias=0.625, scale=-1.0
                        )
                        nc.vector.tensor_tensor(bt[:, sl], ct[:, sl], et[:, sl], OP.mult)

                    if pending is not None:
                        emit_tail(pending)
                    pending = (tci, o, g, at, bt, S)
        emit_tail(pending)
        flush_evac()

    nc.compile()
    return nc


_NC_CACHE: list = []


def _to_bf16(a: np.ndarray) -> np.ndarray:
    import ml_dtypes

    return np.ascontiguousarray(a.astype(ml_dtypes.bfloat16))


def build_in_maps(u_pre: np.ndarray, k_syn: np.ndarray, e_syn: np.ndarray) -> list:
    import ml_dtypes

    eye = np.eye(128, dtype=ml_dtypes.bfloat16)
    neye = (-np.eye(128)).astype(ml_dtypes.bfloat16)
    in_maps = []
    for i in range(N_CORES):
        lo, hi = i * NL, (i + 1) * NL
        in_maps.append(
            {
                "u": _to_bf16(u_pre[lo:hi, 0, :]),
                "k": _to_bf16(k_syn[lo:hi]),
                "e": _to_bf16(e_syn[lo:hi]),
                "ident": eye,
                "nident": neye,
            }
        )
    return in_maps


def kernel(u_pre: np.ndarray, k_syn: np.ndarray, e_syn: np.ndarray) -> np.ndarray:
    if not _NC_CACHE:
        _NC_CACHE.append(_build_nc())
    nc = _NC_CACHE[0]

    in_maps = build_in_maps(u_pre, k_syn, e_syn)
    res = run_bass_kernel_spmd(nc, in_maps, list(range(N_CORES)))
    partials = np.stack([res.results[i]["out"] for i in range(N_CORES)])
    return partials.sum(axis=0, dtype=np.float32)


# revision 27
# speedup vs baseline: 1.0072x; 1.0072x over previous
"""Trainium2 Bass kernel for nn_NonSpikingOutput.

Reference semantics (N=4096 neurons, O=3 outputs, T=4096 steps):
    g = k/(e-k); act = clip(u, 0, 1); RK2 with i_syn frozen collapses to
        v_t = a_t * v_{t-1} + b_t
        a_t = 0.625 - 0.075*act*g,  b_t = 0.075*act*g*e = (0.625 - a_t)*e
    out[o, t] = sum_n v[n, o, t]

Final design (v8; every choice below is HW-measured, see git-less backups
kernel_v3/v4/v6_backup.py for the ancestry — 319us -> 253 -> 236us):
  - Inputs uploaded as bf16 (host truncation): halves HBM traffic and makes
    every DVE tensor_tensor eligible for the 2x perf mode.
  - d = e-k computed on the PE: psum_d = I@e + (-I)@k (identity stationaries
    uploaded as host constants). ACT Ln reads the f32 psum directly.
  - 0.075 folded into the ACT Exp bias: h = exp(-ln(e-k) + ln 0.075)
    = 0.075/(e-k), so c = act*k*h, a = 0.625 - c (ACT Copy), b = c*e (DVE),
    and the scan yields v directly -- no rescale.
  - DVE carries only: clip (TS 4x), t=k*h, c=t*act, b=c*e (TT 2x bf16), scan.
  - Software pipelining: scan/carry/colsum for tile i-1 are emitted during
    tile i, so the c(i)->a(i)->scan(i) ACT round trip never stalls DVE.
  - PSUM: d tile (128,2048)f32 = 4 banks (bufs=1) + colsum row (1,2048)f32 =
    4 banks (bufs=1) -- exactly 8 banks.
  - No GPSIMD (concurrent GPSIMD inflates DVE TT 4.5x via SBUF port sharing).

Measured engine budget per core (236us wall): DVE 200us busy (scan 106.8 +
3 TT 87.5 + clips 5.5 -- all at their per-mode throughput ceilings), ACT
171us, PE 193us, DMA ~100us. Rejected variants (measured worse): scan/a
halving (+overhead > stall savings), u-prefetch (DMA-bandwidth bound),
STT fusion (no DVE fast mode), GPSIMD offload (poisons DVE).

Sharding: neuron dim N split across 8 cores (512 each); host sums the
per-core (O, T) partials.
"""

import sys
from contextlib import ExitStack

import numpy as np

sys.path.insert(0, "/opt/trn_rl_repo")

import concourse.bass as bass
import concourse.tile as tile
from concourse import bacc, mybir
from concourse.bass_utils import run_bass_kernel_spmd

N_CORES = 8
N, O, T = 4096, 3, 4096
NL = N // N_CORES  # neurons per core
NG = NL // 128     # 128-partition neuron groups per core
F = 2048           # time-chunk (free dim) per tile
TC = T // F
FP32 = mybir.dt.float32
BF16 = mybir.dt.bfloat16
OP = mybir.AluOpType
AF = mybir.ActivationFunctionType

LN_0075 = float(np.log(0.075))  # Exp bias: exp(-lnd + ln 0.075) = 0.075/d


def _build_nc() -> bass.Bass:
    nc = bacc.Bacc(
        "TRN2", target_bir_lowering=False, debug=False, num_devices=N_CORES
    )
    u = nc.dram_tensor("u", [NL, T], BF16, kind="ExternalInput")
    k = nc.dram_tensor("k", [NL, O, T], BF16, kind="ExternalInput")
    e = nc.dram_tensor("e", [NL, O, T], BF16, kind="ExternalInput")
    ident_d = nc.dram_tensor("ident", [128, 128], BF16, kind="ExternalInput")
    nident_d = nc.dram_tensor("nident", [128, 128], BF16, kind="ExternalInput")
    out = nc.dram_tensor("out", [O, T], FP32, kind="ExternalOutput")

    with tile.TileContext(nc) as tc, ExitStack() as ctx:
        # Preload the ACT table set holding Ln+Exp+Copy (set 6) once.
        preload = mybir.InstLoadActFuncSet(
            name=nc.get_next_instruction_name(), act_func_set_id=6, ins=[], outs=[]
        )
        nc.scalar.add_instruction(preload)

        const_pool = ctx.enter_context(tc.tile_pool(name="const", bufs=1))
        ones = const_pool.tile([128, 1], BF16)
        nc.vector.memset(ones[:], 1.0)
        exp_bias = const_pool.tile([128, 1], FP32)
        nc.vector.memset(exp_bias[:], LN_0075)
        ident = const_pool.tile([128, 128], BF16)
        nc.sync.dma_start(ident[:], ident_d[:, :])
        nident = const_pool.tile([128, 128], BF16)
        nc.sync.dma_start(nident[:], nident_d[:, :])
        # one carry column per (o, g): column o*NG+g
        carry = const_pool.tile([128, O * NG], FP32)

        u_pool = ctx.enter_context(tc.tile_pool(name="u", bufs=2))
        act_pool = ctx.enter_context(tc.tile_pool(name="act", bufs=NG + 1))
        k_pool = ctx.enter_context(tc.tile_pool(name="k", bufs=3))
        e_pool = ctx.enter_context(tc.tile_pool(name="e", bufs=3))
        l_pool = ctx.enter_context(tc.tile_pool(name="l", bufs=3))
        h_pool = ctx.enter_context(tc.tile_pool(name="h", bufs=3))
        t_pool = ctx.enter_context(tc.tile_pool(name="t", bufs=3))
        c_pool = ctx.enter_context(tc.tile_pool(name="c", bufs=3))
        a_pool = ctx.enter_context(tc.tile_pool(name="a", bufs=3))
        b_pool = ctx.enter_context(tc.tile_pool(name="b", bufs=3))
        w_pool = ctx.enter_context(tc.tile_pool(name="w", bufs=3))
        r_pool = ctx.enter_context(tc.tile_pool(name="r", bufs=2))
        ps_pool = ctx.enter_context(tc.tile_pool(name="ps", bufs=1, space="PSUM"))
        d_pool = ctx.enter_context(tc.tile_pool(name="d", bufs=1, space="PSUM"))

        acts: dict[int, object] = {}
        ps_by_to: dict[tuple, object] = {}
        pending = None  # (tci, o, g, at, bt)
        evac_stash: list = []  # [(tci, o, ps)] deferred one tile

        def flush_evac():
            """Deferred psum evacuation: emitted one tile after the group
            closes so the a-Copy of the current tile precedes it in ACT's
            in-order queue (evac ahead of `a` stalled the scan ~2us per
            group boundary). It still precedes the next ps alloc (bufs=1)."""
            while evac_stash:
                tci, o, ps = evac_stash.pop(0)
                t0 = tci * F
                row = r_pool.tile([1, F], FP32, tag="row")
                nc.scalar.copy(row[:], ps[:])
                nc.sync.dma_start(out[o : o + 1, t0 : t0 + F], row[:, :])

        def emit_tail(item):
            """scan + carry + colsum for a finished front-end tile."""
            tci, o, g, at, bt, S = item
            wt = w_pool.tile([128, F], BF16, tag="w")
            ci = o * NG + g
            init = 0.0 if tci == 0 else carry[:, ci : ci + 1]
            C = F // S
            for s in range(S):
                sl = slice(s * C, (s + 1) * C)
                sub_init = init if s == 0 else wt[:, s * C - 1 : s * C]
                nc.vector.tensor_tensor_scan(
                    wt[:, sl], at[:, sl], bt[:, sl], sub_init, OP.mult, OP.add
                )
            if tci < TC - 1:
                nc.scalar.copy(carry[:, ci : ci + 1], wt[:, F - 1 : F])
            flush_evac()
            if g == 0:
                ps_by_to[(tci, o)] = ps_pool.tile(
                    [1, F], FP32, tag="ps", name=f"ps{tci}_{o}"
                )
            ps = ps_by_to[(tci, o)]
            for s in range(F // 512):
                nc.tensor.matmul(
                    ps[0:1, s * 512 : (s + 1) * 512],
                    ones[:],
                    wt[:, s * 512 : (s + 1) * 512],
                    start=(g == 0),
                    stop=(g == NG - 1),
                )
            if g == NG - 1:
                evac_stash.append((tci, o, ps))

        for tci in range(TC):
            t0 = tci * F
            for o in range(O):
                for g in range(NG):
                    p0 = g * 128
                    kt = k_pool.tile([128, F], BF16, tag="k")
                    et = e_pool.tile([128, F], BF16, tag="e")
                    dma_chunks = 4 if (tci, o, g) == (0, 0, 0) else 1
                    DC = F // dma_chunks
                    for s in range(dma_chunks):
                        sl = slice(s * DC, (s + 1) * DC)
                        nc.sync.dma_start(kt[:, sl], k[p0 : p0 + 128, o, t0 + s * DC : t0 + (s + 1) * DC])
                        nc.sync.dma_start(et[:, sl], e[p0 : p0 + 128, o, t0 + s * DC : t0 + (s + 1) * DC])

                    # u/clip after k/e: the k,e -> PE -> Ln -> Exp chain is the
                    # critical path; the clip is DVE filler work.
                    if o == 0:
                        ut = u_pool.tile([128, F], BF16, tag="u")
                        nc.sync.dma_start(ut[:], u[p0 : p0 + 128, t0 : t0 + F])
                        av = act_pool.tile([128, F], BF16, tag="act")
                        nc.vector.tensor_scalar(av[:], ut[:], 0.0, 1.0, OP.max, OP.min)
                        acts[g] = av
                    act = acts[g]

                    # d = e - k on the PE: per 512-chunk, I@e then (-I)@k
                    dps = d_pool.tile([128, F], FP32, tag="d", name=f"d{tci}_{o}_{g}")
                    for s in range(F // 512):
                        sl = slice(s * 512, (s + 1) * 512)
                        nc.tensor.matmul(
                            dps[:, sl], ident[:], et[:, sl], start=True, stop=False
                        )
                        nc.tensor.matmul(
                            dps[:, sl], nident[:], kt[:, sl], start=False, stop=True
                        )

                    # Prime tile: sub-chunk the whole chain so the first DVE
                    # work starts ~7us earlier (pipeline warmup is otherwise
                    # gated on full-tile PE d + Ln + Exp of tile 0).
                    S = 4 if (tci, o, g) == (0, 0, 0) else 1
                    C = F // S
                    lnd = l_pool.tile([128, F], FP32, tag="lnd")
                    ht = h_pool.tile([128, F], BF16, tag="h")
                    tt = t_pool.tile([128, F], BF16, tag="t")
                    ct = c_pool.tile([128, F], BF16, tag="c")
                    at = a_pool.tile([128, F], BF16, tag="a")
                    bt = b_pool.tile([128, F], BF16, tag="b")
                    for s in range(S):
                        sl = slice(s * C, (s + 1) * C)
                        nc.scalar.activation(lnd[:, sl], dps[:, sl], AF.Ln)
                        nc.scalar.activation(
                            ht[:, sl], lnd[:, sl], AF.Exp, bias=exp_bias[:], scale=-1.0
                        )
                        nc.vector.tensor_tensor(tt[:, sl], kt[:, sl], ht[:, sl], OP.mult)
                        nc.vector.tensor_tensor(ct[:, sl], tt[:, sl], act[:, sl], OP.mult)
                        nc.scalar.activation(
                            at[:, sl], ct[:, sl], AF.Copy, bias=0.625, scale=-1.0
                        )
                        nc.vector.tensor_tensor(bt[:, sl], ct[:, sl], et[:, sl], OP.mult)

                    if pending is not None:
                        emit_tail(pending)
                    pending = (tci, o, g, at, bt, S)
        emit_tail(pending)
        flush_evac()

    nc.compile()
    return nc


_NC_CACHE: list = []


def _to_bf16(a: np.ndarray) -> np.ndarray:
    import ml_dtypes

    return np.ascontiguousarray(a.astype(ml_dtypes.bfloat16))


def build_in_maps(u_pre: np.ndarray, k_syn: np.ndarray, e_syn: np.ndarray) -> list:
    import ml_dtypes

    eye = np.eye(128, dtype=ml_dtypes.bfloat16)
    neye = (-np.eye(128)).astype(ml_dtypes.bfloat16)
    in_maps = []
    for i in range(N_CORES):
        lo, hi = i * NL, (i + 1) * NL
        in_maps.append(
            {
                "u": _to_bf16(u_pre[lo:hi, 0, :]),
                "k": _to_bf16(k_syn[lo:hi]),
                "e": _to_bf16(e_syn[lo:hi]),
                "ident": eye,
                "nident": neye,
            }
        )
    return in_maps


def kernel(u_pre: np.ndarray, k_syn: np.ndarray, e_syn: np.ndarray) -> np.ndarray:
    if not _NC_CACHE:
        _NC_CACHE.append(_build_nc())
    nc = _NC_CACHE[0]

    in_maps = build_in_maps(u_pre, k_syn, e_syn)
    res = run_bass_kernel_spmd(nc, in_maps, list(range(N_CORES)))
    partials = np.stack([res.results[i]["out"] for i in range(N_CORES)])
    return partials.sum(axis=0, dtype=np.float32)


# revision 29
# speedup vs baseline: 1.0224x; 1.0151x over previous
"""Trainium2 Bass kernel for nn_NonSpikingOutput.

Reference semantics (N=4096 neurons, O=3 outputs, T=4096 steps):
    g = k/(e-k); act = clip(u, 0, 1); RK2 with i_syn frozen collapses to
        v_t = a_t * v_{t-1} + b_t
        a_t = 0.625 - 0.075*act*g,  b_t = 0.075*act*g*e = (0.625 - a_t)*e
    out[o, t] = sum_n v[n, o, t]

Final design (v8; every choice below is HW-measured, see git-less backups
kernel_v3/v4/v6_backup.py for the ancestry — 319us -> 253 -> 236us):
  - Inputs uploaded as bf16 (host truncation): halves HBM traffic and makes
    every DVE tensor_tensor eligible for the 2x perf mode.
  - d = e-k computed on the PE: psum_d = I@e + (-I)@k (identity stationaries
    uploaded as host constants). ACT Ln reads the f32 psum directly.
  - 0.075 folded into the ACT Exp bias: h = exp(-ln(e-k) + ln 0.075)
    = 0.075/(e-k), so c = act*k*h, a = 0.625 - c (ACT Copy), b = c*e (DVE),
    and the scan yields v directly -- no rescale.
  - DVE carries only: clip (TS 4x), t=k*h, c=t*act, b=c*e (TT 2x bf16), scan.
  - Software pipelining: scan/carry/colsum for tile i-1 are emitted during
    tile i, so the c(i)->a(i)->scan(i) ACT round trip never stalls DVE.
  - PSUM: d tile (128,2048)f32 = 4 banks (bufs=1) + colsum row (1,2048)f32 =
    4 banks (bufs=1) -- exactly 8 banks.
  - No GPSIMD (concurrent GPSIMD inflates DVE TT 4.5x via SBUF port sharing).

Measured engine budget per core (236us wall): DVE 200us busy (scan 106.8 +
3 TT 87.5 + clips 5.5 -- all at their per-mode throughput ceilings), ACT
171us, PE 193us, DMA ~100us. Rejected variants (measured worse): scan/a
halving (+overhead > stall savings), u-prefetch (DMA-bandwidth bound),
STT fusion (no DVE fast mode), GPSIMD offload (poisons DVE).

Sharding: neuron dim N split across 8 cores (512 each); host sums the
per-core (O, T) partials.
"""

import sys
from contextlib import ExitStack

import numpy as np

sys.path.insert(0, "/opt/trn_rl_repo")

import concourse.bass as bass
import concourse.tile as tile
from concourse import bacc, mybir
from concourse.bass_utils import run_bass_kernel_spmd

N_CORES = 8
N, O, T = 4096, 3, 4096
NL = N // N_CORES  # neurons per core
NG = NL // 128     # 128-partition neuron groups per core
F = 2048           # time-chunk (free dim) per tile
TC = T // F
FP32 = mybir.dt.float32
BF16 = mybir.dt.bfloat16
OP = mybir.AluOpType
AF = mybir.ActivationFunctionType

LN_0075 = float(np.log(0.075))  # Exp bias: exp(-lnd + ln 0.075) = 0.075/d


def _build_nc() -> bass.Bass:
    nc = bacc.Bacc(
        "TRN2", target_bir_lowering=False, debug=False, num_devices=N_CORES
    )
    u = nc.dram_tensor("u", [NL, T], BF16, kind="ExternalInput")
    k = nc.dram_tensor("k", [NL, O, T], BF16, kind="ExternalInput")
    e = nc.dram_tensor("e", [NL, O, T], BF16, kind="ExternalInput")
    ident_d = nc.dram_tensor("ident", [128, 128], BF16, kind="ExternalInput")
    nident_d = nc.dram_tensor("nident", [128, 128], BF16, kind="ExternalInput")
    out = nc.dram_tensor("out", [O, T], FP32, kind="ExternalOutput")

    with tile.TileContext(nc) as tc, ExitStack() as ctx:
        # Preload the ACT table set holding Ln+Exp+Copy (set 6) once.
        preload = mybir.InstLoadActFuncSet(
            name=nc.get_next_instruction_name(), act_func_set_id=6, ins=[], outs=[]
        )
        nc.scalar.add_instruction(preload)

        const_pool = ctx.enter_context(tc.tile_pool(name="const", bufs=1))
        ones = const_pool.tile([128, 1], BF16)
        nc.vector.memset(ones[:], 1.0)
        exp_bias = const_pool.tile([128, 1], FP32)
        nc.vector.memset(exp_bias[:], LN_0075)
        ident = const_pool.tile([128, 128], BF16)
        nc.sync.dma_start(ident[:], ident_d[:, :])
        nident = const_pool.tile([128, 128], BF16)
        nc.sync.dma_start(nident[:], nident_d[:, :])
        # one carry column per (o, g): column o*NG+g
        carry = const_pool.tile([128, O * NG], FP32)

        u_pool = ctx.enter_context(tc.tile_pool(name="u", bufs=2))
        act_pool = ctx.enter_context(tc.tile_pool(name="act", bufs=NG + 1))
        k_pool = ctx.enter_context(tc.tile_pool(name="k", bufs=3))
        e_pool = ctx.enter_context(tc.tile_pool(name="e", bufs=3))
        l_pool = ctx.enter_context(tc.tile_pool(name="l", bufs=3))
        h_pool = ctx.enter_context(tc.tile_pool(name="h", bufs=3))
        t_pool = ctx.enter_context(tc.tile_pool(name="t", bufs=3))
        c_pool = ctx.enter_context(tc.tile_pool(name="c", bufs=3))
        a_pool = ctx.enter_context(tc.tile_pool(name="a", bufs=3))
        b_pool = ctx.enter_context(tc.tile_pool(name="b", bufs=3))
        w_pool = ctx.enter_context(tc.tile_pool(name="w", bufs=3))
        r_pool = ctx.enter_context(tc.tile_pool(name="r", bufs=2))
        ps_pool = ctx.enter_context(tc.tile_pool(name="ps", bufs=1, space="PSUM"))
        d_pool = ctx.enter_context(tc.tile_pool(name="d", bufs=1, space="PSUM"))

        acts: dict[int, object] = {}
        ps_by_to: dict[tuple, object] = {}
        pending = None  # (tci, o, g, at, bt)
        evac_stash: list = []  # [(tci, o, ps)] deferred one tile

        def flush_evac():
            """Deferred psum evacuation: emitted one tile after the group
            closes so the a-Copy of the current tile precedes it in ACT's
            in-order queue (evac ahead of `a` stalled the scan ~2us per
            group boundary). It still precedes the next ps alloc (bufs=1)."""
            while evac_stash:
                tci, o, ps = evac_stash.pop(0)
                t0 = tci * F
                row = r_pool.tile([1, F], FP32, tag="row")
                nc.scalar.copy(row[:], ps[:])
                nc.sync.dma_start(out[o : o + 1, t0 : t0 + F], row[:, :])

        def emit_tail(item):
            """scan + carry + colsum for a finished front-end tile."""
            tci, o, g, at, bt, S = item
            wt = w_pool.tile([128, F], BF16, tag="w")
            ci = o * NG + g
            init = 0.0 if tci == 0 else carry[:, ci : ci + 1]
            C = F // S
            for s in range(S):
                sl = slice(s * C, (s + 1) * C)
                sub_init = init if s == 0 else wt[:, s * C - 1 : s * C]
                nc.vector.tensor_tensor_scan(
                    wt[:, sl], at[:, sl], bt[:, sl], sub_init, OP.mult, OP.add
                )
            if tci < TC - 1:
                # carry on DVE (tiny 1-elem op): keeps the ACT queue free of
                # 12 insertions that delay the next tile's Ln/Exp.
                nc.vector.tensor_scalar(
                    carry[:, ci : ci + 1], wt[:, F - 1 : F], 1.0, None, OP.mult
                )
            flush_evac()
            if g == 0:
                ps_by_to[(tci, o)] = ps_pool.tile(
                    [1, F], FP32, tag="ps", name=f"ps{tci}_{o}"
                )
            ps = ps_by_to[(tci, o)]
            for s in range(F // 512):
                nc.tensor.matmul(
                    ps[0:1, s * 512 : (s + 1) * 512],
                    ones[:],
                    wt[:, s * 512 : (s + 1) * 512],
                    start=(g == 0),
                    stop=(g == NG - 1),
                )
            if g == NG - 1:
                evac_stash.append((tci, o, ps))

        for tci in range(TC):
            t0 = tci * F
            for o in range(O):
                for g in range(NG):
                    p0 = g * 128
                    kt = k_pool.tile([128, F], BF16, tag="k")
                    et = e_pool.tile([128, F], BF16, tag="e")
                    dma_chunks = 4 if (tci, o, g) == (0, 0, 0) else 1
                    DC = F // dma_chunks
                    for s in range(dma_chunks):
                        sl = slice(s * DC, (s + 1) * DC)
                        nc.sync.dma_start(kt[:, sl], k[p0 : p0 + 128, o, t0 + s * DC : t0 + (s + 1) * DC])
                        nc.sync.dma_start(et[:, sl], e[p0 : p0 + 128, o, t0 + s * DC : t0 + (s + 1) * DC])

                    # u/clip after k/e: the k,e -> PE -> Ln -> Exp chain is the
                    # critical path; the clip is DVE filler work.
                    if o == 0:
                        ut = u_pool.tile([128, F], BF16, tag="u")
                        nc.sync.dma_start(ut[:], u[p0 : p0 + 128, t0 : t0 + F])
                        av = act_pool.tile([128, F], BF16, tag="act")
                        nc.vector.tensor_scalar(av[:], ut[:], 0.0, 1.0, OP.max, OP.min)
                        acts[g] = av
                    act = acts[g]

                    # d = e - k on the PE: per 512-chunk, I@e then (-I)@k
                    dps = d_pool.tile([128, F], FP32, tag="d", name=f"d{tci}_{o}_{g}")
                    for s in range(F // 512):
                        sl = slice(s * 512, (s + 1) * 512)
                        nc.tensor.matmul(
                            dps[:, sl], ident[:], et[:, sl], start=True, stop=False
                        )
                        nc.tensor.matmul(
                            dps[:, sl], nident[:], kt[:, sl], start=False, stop=True
                        )

                    # Prime tile: sub-chunk the whole chain so the first DVE
                    # work starts ~7us earlier (pipeline warmup is otherwise
                    # gated on full-tile PE d + Ln + Exp of tile 0).
                    S = 4 if (tci, o, g) == (0, 0, 0) else 1
                    C = F // S
                    lnd = l_pool.tile([128, F], FP32, tag="lnd")
                    ht = h_pool.tile([128, F], BF16, tag="h")
                    tt = t_pool.tile([128, F], BF16, tag="t")
                    ct = c_pool.tile([128, F], BF16, tag="c")
                    at = a_pool.tile([128, F], BF16, tag="a")
                    bt = b_pool.tile([128, F], BF16, tag="b")
                    for s in range(S):
                        sl = slice(s * C, (s + 1) * C)
                        nc.scalar.activation(lnd[:, sl], dps[:, sl], AF.Ln)
                        nc.scalar.activation(
                            ht[:, sl], lnd[:, sl], AF.Exp, bias=exp_bias[:], scale=-1.0
                        )
                        nc.vector.tensor_tensor(tt[:, sl], kt[:, sl], ht[:, sl], OP.mult)
                        nc.vector.tensor_tensor(ct[:, sl], tt[:, sl], act[:, sl], OP.mult)
                        if g == 0:
                            # group-boundary tiles: a on DVE (TS 4x, 0.67us)
                            # so the scan never waits on a cold ACT queue.
                            nc.vector.tensor_scalar(
                                at[:, sl], ct[:, sl], -1.0, 0.625, OP.mult, OP.add
                            )
                        else:
                            nc.scalar.activation(
                                at[:, sl], ct[:, sl], AF.Copy, bias=0.625, scale=-1.0
                            )
                        nc.vector.tensor_tensor(bt[:, sl], ct[:, sl], et[:, sl], OP.mult)

                    if pending is not None:
                        emit_tail(pending)
                    pending = (tci, o, g, at, bt, S)
        emit_tail(pending)
        flush_evac()

    nc.compile()
    return nc


_NC_CACHE: list = []


def _to_bf16(a: np.ndarray) -> np.ndarray:
    import ml_dtypes

    return np.ascontiguousarray(a.astype(ml_dtypes.bfloat16))


def build_in_maps(u_pre: np.ndarray, k_syn: np.ndarray, e_syn: np.ndarray) -> list:
    import ml_dtypes

    eye = np.eye(128, dtype=ml_dtypes.bfloat16)
    neye = (-np.eye(128)).astype(ml_dtypes.bfloat16)
    in_maps = []
    for i in range(N_CORES):
        lo, hi = i * NL, (i + 1) * NL
        in_maps.append(
            {
                "u": _to_bf16(u_pre[lo:hi, 0, :]),
                "k": _to_bf16(k_syn[lo:hi]),
                "e": _to_bf16(e_syn[lo:hi]),
                "ident": eye,
                "nident": neye,
            }
        )
    return in_maps


def kernel(u_pre: np.ndarray, k_syn: np.ndarray, e_syn: np.ndarray) -> np.ndarray:
    if not _NC_CACHE:
        _NC_CACHE.append(_build_nc())
    nc = _NC_CACHE[0]

    in_maps = build_in_maps(u_pre, k_syn, e_syn)
    res = run_bass_kernel_spmd(nc, in_maps, list(range(N_CORES)))
    partials = np.stack([res.results[i]["out"] for i in range(N_CORES)])
    return partials.sum(axis=0, dtype=np.float32)
